# revision 1
# baseline (speedup 1.0000x reference)
"""Trainium2 Bass kernel for nn_ExchangeBlock (segment_reduce family).

Reference computation (per row, N=500k rows):
    contr = (ev*ev) @ P              # [24] -> [4] per-degree sum of squares
    y = concat([x, contr]) @ W + b   # [136] @ [136,136]
    cx = y[:, :132]
    out2 = (y[:, 132:136] @ P.T) * ev

Kernel strategy (pure data parallel over 8 NeuronCores, rows sharded):
  * Both P contractions are folded into host-precomputed weights:
        y' = [x, ev*ev, 1] @ Wfull      (Wfull: [157, 156] fp16)
    where columns 132:156 of Wfull are pre-multiplied by P.T, rows 132:156
    pre-multiplied by P, and row 156 carries the bias. So on-chip work is a
    single 157x156 matmul per row block plus an elementwise gate by ev.
  * 128-row slots; per slot the PE transposes x (and the [x_hi|ev^2|1] tail)
    into feature-major layout (PE->PSUM), engines copy back to SBUF, two
    accumulating matmuls produce y' [128, 156] fp32 in PSUM.
  * fp16 inputs to the PE (fp32 matmul is 4x slower); fp32 accumulate.
  * 1024-row macro tiles give 4KB-contiguous DMA descriptors (partition p
    holds rows 8p..8p+7 of the macro). SWDGE DMAs cast fp32<->fp16 in flight.
"""

import numpy as np

import concourse.bacc as bacc
import concourse.bass as bass
import concourse.mybir as mybir
import concourse.tile as tile
from concourse import bass_utils
from concourse.masks import make_identity

N_TOTAL = 500_000
F = 132                 # x features
M_TOT = 24              # ev components
NUM_DEG = 4
D_IN = F + NUM_DEG      # 136
D_OUT = F + M_TOT       # 156 fused output width (cx | rep)
K_FULL = F + M_TOT + 1  # 157 contraction rows (x | ev^2 | ones)
N_CORES = 8
ROWS_PER_CORE = N_TOTAL // N_CORES  # 62500
A_PER_P = 8
MACRO = 128 * A_PER_P   # 1024 rows per macro tile

REPEATS = [3, 5, 7, 9]
SEG_IDS = np.repeat(np.arange(NUM_DEG), REPEATS)
P_MAT = np.eye(NUM_DEG, dtype=np.float32)[SEG_IDS]  # [24, 4]

FP16 = mybir.dt.float16
FP32 = mybir.dt.float32


def _emit_slot(nc, pools, x_slot, tail_slot, ev_slot, cx_out, g_out, wa, wb, ident):
    """Emit one 128-row slot: transpose -> matmul -> epilogue.

    x_slot:  [128, 132] fp16 SBUF AP (rows on partitions)
    tail_slot: [128, 29] fp16 SBUF AP ([x128:132 | ev^2 | 1])
    ev_slot: [128, 24] fp32 SBUF AP
    cx_out:  [128, 132] fp16 SBUF AP destination
    g_out:   [128, 24] fp32 SBUF AP destination
    """
    ps_t, ps_y, sb_t = pools

    # Transpose x features 0:128 -> [128k, 128r], and tail -> [29k, 128r]
    xt_ps = ps_t.tile([128, 128], FP16, tag="xtp")
    nc.tensor.transpose(xt_ps, x_slot[:, 0:128], ident)
    xt = sb_t.tile([128, 128], FP16, tag="xt")
    nc.vector.tensor_copy(out=xt, in_=xt_ps)

    tl_ps = ps_t.tile([29, 128], FP16, tag="tlp")
    nc.tensor.transpose(tl_ps, tail_slot, ident)
    tl = sb_t.tile([29, 128], FP16, tag="tl")
    nc.scalar.copy(out=tl, in_=tl_ps)

    y_ps = ps_y.tile([128, D_OUT], FP32, tag="y")
    nc.tensor.matmul(y_ps, lhsT=xt, rhs=wa, start=True, stop=False)
    nc.tensor.matmul(y_ps, lhsT=tl, rhs=wb, start=False, stop=True)

    # cx = y[:, 0:132] (fp16 staging, DMA upcasts on store)
    nc.scalar.copy(out=cx_out, in_=y_ps[:, 0:F])
    # gated = y[:, 132:156] * ev   (exact fp32 ev)
    nc.vector.tensor_mul(out=g_out, in0=y_ps[:, F:D_OUT], in1=ev_slot)


def build_module(rows=ROWS_PER_CORE):
    """Build + compile the per-core Bass module. Same program on all cores."""
    n_macro = rows // MACRO
    tail_rows = rows - n_macro * MACRO
    assert tail_rows == 0 or tail_rows >= 1

    nc = bacc.Bacc("TRN2", num_devices=N_CORES)
    x_d = nc.dram_tensor("x", [rows, F], FP32, kind="ExternalInput")
    ev_d = nc.dram_tensor("ev", [rows, M_TOT], FP32, kind="ExternalInput")
    wf_d = nc.dram_tensor("wfull", [K_FULL, D_OUT], FP16, kind="ExternalInput")
    cx_d = nc.dram_tensor("cx", [rows, F], FP32, kind="ExternalOutput")
    g_d = nc.dram_tensor("gated", [rows, M_TOT], FP32, kind="ExternalOutput")

    with tile.TileContext(nc) as tc:
        with (
            tc.tile_pool(name="const", bufs=1) as const,
            tc.tile_pool(name="xin", bufs=2) as xpool,
            tc.tile_pool(name="evin", bufs=2) as evpool,
            tc.tile_pool(name="tail", bufs=2) as tailpool,
            tc.tile_pool(name="lhs", bufs=4) as sb_t,
            tc.tile_pool(name="cxout", bufs=2) as cxpool,
            tc.tile_pool(name="gout", bufs=2) as gpool,
            tc.tile_pool(name="ps_t", bufs=2, space="PSUM") as ps_t,
            tc.tile_pool(name="ps_y", bufs=4, space="PSUM") as ps_y,
        ):
            ident = const.tile([128, 128], FP16)
            make_identity(nc, ident)
            wa = const.tile([128, D_OUT], FP16)
            nc.sync.dma_start(out=wa, in_=wf_d.ap()[0:128, :])
            wb = const.tile([K_FULL - 128, D_OUT], FP16)
            nc.sync.dma_start(out=wb, in_=wf_d.ap()[128:K_FULL, :])

            pools = (ps_t, ps_y, sb_t)

            for m in range(n_macro):
                r0 = m * MACRO
                x_sb = xpool.tile([128, A_PER_P, F], FP16, tag="x")
                nc.gpsimd.dma_start(
                    out=x_sb,
                    in_=x_d.ap()[r0:r0 + MACRO, :].rearrange(
                        "(p a) k -> p a k", p=128),
                )
                ev_sb = evpool.tile([128, A_PER_P, M_TOT], FP32, tag="ev")
                nc.sync.dma_start(
                    out=ev_sb,
                    in_=ev_d.ap()[r0:r0 + MACRO, :].rearrange(
                        "(p a) k -> p a k", p=128),
                )
                tail = tailpool.tile([128, A_PER_P, 29], FP16, tag="tail")
                nc.vector.tensor_copy(out=tail[:, :, 0:4], in_=x_sb[:, :, 128:F])
                nc.vector.tensor_mul(out=tail[:, :, 4:28], in0=ev_sb, in1=ev_sb)
                nc.vector.memset(tail[:, :, 28:29], 1.0)

                cx_sb = cxpool.tile([128, A_PER_P, F], FP16, tag="cx")
                g_sb = gpool.tile([128, A_PER_P, M_TOT], FP32, tag="g")
                for a in range(A_PER_P):
                    _emit_slot(nc, pools, x_sb[:, a, :], tail[:, a, :],
                               ev_sb[:, a, :], cx_sb[:, a, :], g_sb[:, a, :],
                               wa, wb, ident)
                nc.gpsimd.dma_start(
                    out=cx_d.ap()[r0:r0 + MACRO, :].rearrange(
                        "(p a) k -> p a k", p=128),
                    in_=cx_sb,
                )
                nc.sync.dma_start(
                    out=g_d.ap()[r0:r0 + MACRO, :].rearrange(
                        "(p a) k -> p a k", p=128),
                    in_=g_sb,
                )

            if tail_rows:
                # Process the final 128 rows (overlapping already-covered rows);
                # store only the last tail_rows rows.
                r0 = rows - 128
                keep = 128 - tail_rows  # partitions [keep:128] are stored
                x_sb = xpool.tile([128, F], FP16, tag="xt_last")
                nc.gpsimd.dma_start(out=x_sb, in_=x_d.ap()[r0:rows, :])
                ev_sb = evpool.tile([128, M_TOT], FP32, tag="ev_last")
                nc.sync.dma_start(out=ev_sb, in_=ev_d.ap()[r0:rows, :])
                tail = tailpool.tile([128, 29], FP16, tag="tail_last")
                nc.vector.tensor_copy(out=tail[:, 0:4], in_=x_sb[:, 128:F])
                nc.vector.tensor_mul(out=tail[:, 4:28], in0=ev_sb, in1=ev_sb)
                nc.vector.memset(tail[:, 28:29], 1.0)
                cx_sb = cxpool.tile([128, F], FP16, tag="cx_last")
                g_sb = gpool.tile([128, M_TOT], FP32, tag="g_last")
                _emit_slot(nc, pools, x_sb, tail, ev_sb, cx_sb, g_sb,
                           wa, wb, ident)
                nc.gpsimd.dma_start(
                    out=cx_d.ap()[r0 + keep:rows, :], in_=cx_sb[keep:128, :])
                nc.sync.dma_start(
                    out=g_d.ap()[r0 + keep:rows, :], in_=g_sb[keep:128, :])

    nc.compile()
    return nc


def make_wfull(W, b):
    """Fold both P contractions + bias into a single [157, 156] weight."""
    W = np.asarray(W, np.float32)
    b = np.asarray(b, np.float32)
    wcat = np.concatenate([W[:, :F], W[:, F:] @ P_MAT.T], axis=1)   # [136,156]
    bprime = np.concatenate([b[:F], b[F:] @ P_MAT.T])               # [156]
    wfull = np.concatenate(
        [wcat[:F], P_MAT @ wcat[F:], bprime[None, :]], axis=0)      # [157,156]
    return wfull.astype(np.float16)


_CACHE = {}


def _get_module(rows):
    if rows not in _CACHE:
        _CACHE[rows] = build_module(rows)
    return _CACHE[rows]


def run(nc, x, ev, wfull, rows):
    """Shard rows over 8 cores, execute, reassemble full outputs."""
    in_maps = []
    for c in range(N_CORES):
        sl = slice(c * rows, (c + 1) * rows)
        in_maps.append({"x": x[sl], "ev": ev[sl], "wfull": wfull})
    res = bass_utils.run_bass_kernel_spmd(
        nc, in_maps, core_ids=list(range(N_CORES)))
    cx = np.concatenate([res.results[c]["cx"] for c in range(N_CORES)], axis=0)
    g = np.concatenate([res.results[c]["gated"] for c in range(N_CORES)], axis=0)
    return cx, g


def kernel(x, ev, W, b):
    x = np.ascontiguousarray(np.asarray(x, np.float32))
    ev = np.ascontiguousarray(np.asarray(ev, np.float32))
    assert x.shape == (N_TOTAL, F) and ev.shape == (N_TOTAL, M_TOT)
    wfull = make_wfull(W, b)
    nc = _get_module(ROWS_PER_CORE)
    cx, g = run(nc, x, ev, wfull, ROWS_PER_CORE)
    return cx, g
